# revision 2
# baseline (speedup 1.0000x reference)
"""CoverageLoss (histogram binning) Trainium2 kernel.

Computes WEIGHT * mean(1 - occupancy) where occupancy[n] is the fraction of
64 angular-histogram bins of atan2(c_seq[n,:,1], c_seq[n,:,0]) that are
non-empty.

Strategy (8 NeuronCores, data-parallel over rows):
  - Each core gets 256 of the 2048 rows (2 partition-tiles of 128 rows).
  - Only *occupancy* matters (hist > 0), so per row we build bit masks of
    "bin present" and OR-reduce them; the scalar loss is assembled on host
    from popcounts.
  - Angle binning without atan2: with t = y/x, tv = x/y (both via the fast
    DVE reciprocal), the identity
        atan(t) = atan(clip(t,-1,1)) - atan(clip(tv,-1,1)) + sign(t)*pi/4
    holds for every t, so a single select-free fp32 pipeline yields the
    half-circle bin j in [0,32). The x<0 class occupies the other half
    circle; since popcount is invariant under within-class bit bijections,
    two 32-bit masks (x>=0 / x<0 classes) suffice per row.
"""

import sys

sys.path.insert(0, "/opt/trn_rl_repo")

from contextlib import ExitStack

import numpy as np

import concourse.bass as bass  # noqa: F401  (AP types come through tile/bacc)
import concourse.tile as tile
from concourse import bacc, bass_utils, mybir

# Problem constants (hardcoded per the harness contract).
N_ROWS = 2048
T = 4096
N_CORES = 8
ROWS_PER_CORE = N_ROWS // N_CORES  # 256
P = 128
ROW_TILES = ROWS_PER_CORE // P  # 2
CHUNK = 1024  # (x, y) pairs processed per chunk
N_CHUNKS = T // CHUNK  # 4

BINS = 64
HI = 3.14159265
LO = -HI
W_BIN = (HI - LO) / BINS
WEIGHT = 1.0

F32 = mybir.dt.float32
I32 = mybir.dt.int32
Alu = mybir.AluOpType
Act = mybir.ActivationFunctionType

_CACHE: dict = {}


def _build_program():
    """Build the per-core Bass program (SPMD: same program, per-core data)."""
    nc = bacc.Bacc(
        "TRN2", target_bir_lowering=False, debug=False, num_devices=N_CORES
    )
    d_in = nc.dram_tensor(
        "c", (ROWS_PER_CORE, 2 * T), F32, kind="ExternalInput"
    ).ap()
    # Output: per partition-row, one int32 occupancy word per (row-tile, class).
    d_out = nc.dram_tensor(
        "masks", (P, 2 * ROW_TILES), I32, kind="ExternalOutput"
    ).ap()

    inv_w = 1.0 / W_BIN

    with tile.TileContext(nc) as tc:
        with ExitStack() as ctx:
            pin = ctx.enter_context(tc.tile_pool(name="pin", bufs=3))
            pw = ctx.enter_context(tc.tile_pool(name="pw", bufs=2))
            pacc = ctx.enter_context(tc.tile_pool(name="pacc", bufs=1))

            accs = []
            for rt in range(ROW_TILES):
                acc_lo = pacc.tile([P, 64], I32, tag=f"acclo{rt}")
                acc_hi = pacc.tile([P, 64], I32, tag=f"acchi{rt}")
                nc.vector.memset(acc_lo[:], 0)
                nc.vector.memset(acc_hi[:], 0)
                accs.append((acc_lo, acc_hi))

            for rt in range(ROW_TILES):
                acc_lo, acc_hi = accs[rt]
                for chi in range(N_CHUNKS):
                    tin = pin.tile([P, 2 * CHUNK], F32, tag="in")
                    nc.sync.dma_start(
                        tin[:],
                        d_in[
                            rt * P : (rt + 1) * P,
                            chi * 2 * CHUNK : (chi + 1) * 2 * CHUNK,
                        ],
                    )
                    pairs = tin[:].rearrange("p (n two) -> p n two", two=2)
                    xv = pairs[:, :, 0]
                    yv = pairs[:, :, 1]

                    rx = pw.tile([P, CHUNK], F32, tag="rx")
                    nc.vector.reciprocal_approx_fast(rx[:], xv)
                    ry = pw.tile([P, CHUNK], F32, tag="ry")
                    nc.vector.reciprocal_approx_fast(ry[:], yv)

                    t = pw.tile([P, CHUNK], F32, tag="t")
                    nc.gpsimd.tensor_tensor(t[:], yv, rx[:], Alu.mult)
                    tv = pw.tile([P, CHUNK], F32, tag="tv")
                    nc.gpsimd.tensor_tensor(tv[:], xv, ry[:], Alu.mult)

                    tc_ = pw.tile([P, CHUNK], F32, tag="tc")
                    nc.vector.tensor_scalar(
                        tc_[:], t[:], -1.0, 1.0, Alu.max, Alu.min
                    )
                    tvc = pw.tile([P, CHUNK], F32, tag="tvc")
                    nc.vector.tensor_scalar(
                        tvc[:], tv[:], -1.0, 1.0, Alu.max, Alu.min
                    )

                    a1 = pw.tile([P, CHUNK], F32, tag="a1")
                    nc.scalar.activation(a1[:], tc_[:], Act.Arctan)
                    a2 = pw.tile([P, CHUNK], F32, tag="a2")
                    nc.scalar.activation(a2[:], tvc[:], Act.Arctan)

                    dd = pw.tile([P, CHUNK], F32, tag="dd")
                    nc.vector.tensor_tensor(dd[:], a1[:], a2[:], Alu.subtract)

                    # jD = floor(D/w + 24) in [16, 32); the HW ACT f32->i32
                    # convert rounds to nearest-even, so bias 23.5 gives floor.
                    jd = pw.tile([P, CHUNK], I32, tag="jd")
                    nc.scalar.activation(
                        jd[:], dd[:], Act.Copy, bias=23.5, scale=inv_w
                    )

                    # s16 = 16*(t < 0); j = jD - s16 in [0, 32)
                    s16 = pw.tile([P, CHUNK], I32, tag="s16")
                    nc.gpsimd.tensor_scalar(
                        s16[:], t[:], 0.0, 16.0, Alu.is_lt, Alu.mult
                    )
                    j = pw.tile([P, CHUNK], I32, tag="j")
                    nc.vector.tensor_tensor(j[:], jd[:], s16[:], Alu.subtract)

                    # class bits from sign(x)
                    sxb = pw.tile([P, CHUNK], I32, tag="sxb")
                    nc.gpsimd.tensor_scalar(sxb[:], xv, 0.0, None, Alu.is_lt)
                    nxb = pw.tile([P, CHUNK], I32, tag="nxb")
                    nc.gpsimd.tensor_scalar(nxb[:], xv, 0.0, None, Alu.is_ge)

                    mhi = pw.tile([P, CHUNK], I32, tag="mhi")
                    nc.vector.tensor_tensor(
                        mhi[:], sxb[:], j[:], Alu.logical_shift_left
                    )
                    mlo = pw.tile([P, CHUNK], I32, tag="mlo")
                    nc.vector.tensor_tensor(
                        mlo[:], nxb[:], j[:], Alu.logical_shift_left
                    )

                    for m, acc in ((mlo, acc_lo), (mhi, acc_hi)):
                        width = CHUNK
                        while width > 64:
                            h = width // 2
                            nc.vector.tensor_tensor(
                                m[:, 0:h], m[:, 0:h], m[:, h:width], Alu.bitwise_or
                            )
                            width = h
                        nc.vector.tensor_tensor(
                            acc[:], acc[:], m[:, 0:64], Alu.bitwise_or
                        )

            # Final fold 64 -> 1 word per (row-tile, class) and store.
            for rt in range(ROW_TILES):
                for cls, acc in enumerate(accs[rt]):
                    width = 64
                    while width > 1:
                        h = width // 2
                        nc.vector.tensor_tensor(
                            acc[:, 0:h], acc[:, 0:h], acc[:, h:width], Alu.bitwise_or
                        )
                        width = h
                    nc.sync.dma_start(
                        d_out[:, 2 * rt + cls : 2 * rt + cls + 1], acc[:, 0:1]
                    )

    nc.compile()
    return nc


def _get_nc():
    if "nc" not in _CACHE:
        _CACHE["nc"] = _build_program()
    return _CACHE["nc"]


def _popcount(a: np.ndarray) -> np.ndarray:
    return np.unpackbits(a.astype(np.uint32).view(np.uint8), axis=-1).sum(
        axis=-1, dtype=np.int64
    )


def _run(c_seq: np.ndarray, trace: bool = False):
    nc = _get_nc()
    c = np.ascontiguousarray(np.asarray(c_seq, dtype=np.float32)).reshape(
        N_CORES, ROWS_PER_CORE, 2 * T
    )
    in_maps = [{"c": c[k]} for k in range(N_CORES)]
    res = bass_utils.run_bass_kernel_spmd(
        nc, in_maps, core_ids=list(range(N_CORES)), trace=trace
    )
    masks = np.stack([r["masks"] for r in res.results])  # (8, 128, 2*ROW_TILES)
    total_occupied = _popcount(masks.reshape(-1, 1)).sum()
    loss = WEIGHT * (1.0 - total_occupied / float(N_ROWS * BINS))
    return np.float32(loss), res


def kernel(**inputs: np.ndarray) -> np.ndarray:
    out, _ = _run(inputs["c_seq"])
    return np.asarray(out, dtype=np.float32)
